# revision 19
# baseline (speedup 1.0000x reference)
"""Trainium2 Bass kernel for nn_ModelLSTM (36-step scalar-feedback LSTM).

Model (per reference):
    emb = relu(x @ W1.T + b1)                       # [B, 511], constant across steps
    x0 = -0.8; h0 = c0 = 0
    step t: inp = [emb, xin]                        # [B, 512]
            gates = inp @ Wih.T + bih + h @ Whh.T + bhh
            i,f,g,o = split(gates); c' = sig(f)*c + sig(i)*tanh(g); h' = sig(o)*tanh(c')
            y = h' @ W3.T + b3 ; xin' = y
    output ys = [36, B, 1]

Restructuring (host-side, exact in fp64):
  * xin folds into the recurrent weights: Whh_eff = Whh + Wih[:,511:] @ W3.
  * emb contribution A = Wih[:,:511] @ emb.T is precomputed once on device,
    kept RESIDENT in SBUF (fp16, x256 scale) and replayed into PSUM per step
    via an identity matmul (DVE-add on a couple of low-urgency tiles to
    balance PE vs DVE). Step-0 is fused into the A-precompute pass.

Mixed precision (validated in sim; ~0.016 rel err vs 2e-2 budget):
  * i/f/o gate matmuls: fp8 e4m3 x16-scaled weights AND h, DoubleRow perf
    mode (2 k-tiles per pass -> half the PE time; measured 214ns/MM).
  * g (tanh) gate matmul: fp8 DoubleRow for k rows 0-255, fp16 for rows
    256-511 (tanh slope 1 makes g 4x more error-sensitive; half-K fp8 keeps
    the error at ~sqrt(1/2) of full fp8).
  * Everything else fp16 (gates, h, c, A, emb); PSUM carries a 256x scale
    which ACT's free activation scale undoes.

Sharding: pure data-parallel over batch (16384 -> 8 cores x 2048). Weights
replicated. No collectives.
"""

import sys

for _p in ("/opt/trn_rl_repo",):
    if _p not in sys.path:
        sys.path.insert(0, _p)

import numpy as np
import ml_dtypes

BF16 = ml_dtypes.bfloat16
FP8 = ml_dtypes.float8_e4m3  # TRN variant (inf at 256); values are tiny
FP16 = np.float16

N_CORES = 8
B = 16384
BL = B // N_CORES  # 2048 batch per core
HID = 512
EMB = 511
STEPS = 36
NG = 4 * HID  # 2048 gate rows
M_TILES = 16  # gate row tiles of 128; m = 0..15, gate type = m//4, hid = m%4
K_TILES = 4  # contraction tiles of 128 over HID
NC_CH = BL // 512  # 4 free-dim chunks of 512
M8 = [0, 1, 2, 3, 4, 5, 6, 7, 12, 13, 14, 15]  # i/f/o m-tiles (full fp8 DoubleRow)
SCL = 16.0  # per-operand fp8/fp16 scale; psum carries SCL^2 = 256


def _build_program(
    b3_val: float,
    n_steps: int = STEPS,
    dve_add_ms: tuple = (6, 7),
    g_halfk: bool = True,
    split_hids: tuple = (0, 1),
    y_on_dve: bool = True,
):
    """Build the Bass program.

    dve_add_ms: m-tiles whose A contribution is added on DVE (psum + A -> z)
      instead of a PE identity matmul; balances PE vs DVE. Pick f-tiles of
      hids 2/3 (m=6,7): most slack before their h is consumed.
    g_halfk: g-gate matmul k rows 0-255 in fp8 DoubleRow, 256-511 in fp16.
    split_hids: hids whose cell update runs in column halves, so their h8
      lands early for the next step's first (kp=0) matmuls.
    """
    import concourse.bass as bass
    import concourse.bacc as bacc
    import concourse.tile as tile
    from concourse import mybir

    fp32 = mybir.dt.float32
    fp16 = mybir.dt.float16
    fp8 = mybir.dt.float8e4
    AF = mybir.ActivationFunctionType
    OP = mybir.AluOpType
    DR = mybir.MatmulPerfMode.DoubleRow

    nc = bacc.Bacc(
        "TRN2",
        target_bir_lowering=False,
        debug=False,
        num_devices=N_CORES,
    )

    # ---- DRAM I/O (per-core shapes) ----
    xT_d = nc.dram_tensor("xT", [24, BL], fp16, kind="ExternalInput")
    w1t_d = nc.dram_tensor("w1t", [24, 512], fp16, kind="ExternalInput")
    w8_d = nc.dram_tensor("w8", [2, 128, 2, 12 * 128], fp8, kind="ExternalInput")
    w8g_d = nc.dram_tensor("w8g", [128, 2, 512], fp8, kind="ExternalInput")
    wg_d = nc.dram_tensor("wg", [HID, 640], fp16, kind="ExternalInput")
    we_d = nc.dram_tensor("we", [HID, NG], fp16, kind="ExternalInput")
    ident_d = nc.dram_tensor("ident", [128, 128], fp16, kind="ExternalInput")
    bias_st_d = nc.dram_tensor("bias_st", [128, M_TILES], fp32, kind="ExternalInput")
    bias_d0_d = nc.dram_tensor("bias_d0", [128, M_TILES], fp32, kind="ExternalInput")
    out_d = nc.dram_tensor("out", [STEPS, BL], fp16, kind="ExternalOutput")

    with tile.TileContext(nc) as tc:
        with (
            tc.tile_pool(name="const", bufs=1) as constp,
            tc.tile_pool(name="apool", bufs=1) as apool,
            tc.tile_pool(name="state", bufs=1) as statep,
            tc.tile_pool(name="hpool", bufs=2) as hpool,
            tc.tile_pool(name="work", bufs=2) as workp,
            tc.tile_pool(name="psum", bufs=2, space=bass.MemorySpace.PSUM) as psump,
        ):
            # ---- load constants ----
            xT_sb = workp.tile([24, BL], fp16, tag="xT", bufs=1, name="xT_sb")
            nc.sync.dma_start(xT_sb[:], xT_d[:])
            w1t_sb = workp.tile([24, 512], fp16, tag="tt", name="w1t_sb")
            nc.sync.dma_start(w1t_sb[:], w1t_d[:])
            w8_sb = []
            for kp in range(2):
                wt = constp.tile([128, 2, 12 * 128], fp8, name=f"w8_{kp}")
                nc.sync.dma_start(wt[:], w8_d[kp])
                w8_sb.append(wt)
            w8g_sb = constp.tile([128, 2, 512], fp8, name="w8g")
            nc.sync.dma_start(w8g_sb[:], w8g_d[:])
            wg_sb = []
            for k in range(K_TILES):
                wt = constp.tile([128, 640], fp16, name=f"wg{k}")
                nc.sync.dma_start(wt[:], wg_d[k * 128 : (k + 1) * 128, :])
                wg_sb.append(wt)
            ident_sb = constp.tile([128, 128], fp16, name="ident")
            nc.sync.dma_start(ident_sb[:], ident_d[:])
            bias_st = constp.tile([128, M_TILES], fp32, name="bias_st")
            nc.sync.dma_start(bias_st[:], bias_st_d[:])
            bias_d0 = constp.tile([128, M_TILES], fp32, name="bias_d0")
            nc.sync.dma_start(bias_d0[:], bias_d0_d[:])

            # we k-tiles parked in the f-gate A slots (a4..a7) - those A tiles
            # are produced LAST in A_ORDER, and the a_stage hop decouples the
            # writeback so the slots recycle without deadlock.
            a_sb = [None] * M_TILES
            we_sb = []
            for k in range(K_TILES):
                wt = apool.tile([128, BL], fp16, tag=f"a{4 + k}", name=f"we{k}")
                nc.sync.dma_start(wt[:], we_d[k * 128 : (k + 1) * 128, :])
                we_sb.append(wt)

            # ---- setup: emb = relu(x @ W1.T + b1) (transposed, fp16) ----
            # emb k-tiles borrow the h16 tags (free during setup; step-0 h16
            # writes land in buf 1 of those tags).
            embT = []
            for mj in range(4):
                eps = psump.tile([128, BL], fp32, tag="gps", name="eps")
                for ncn in range(NC_CH):
                    s = slice(ncn * 512, (ncn + 1) * 512)
                    nc.tensor.matmul(
                        eps[:, s],
                        w1t_sb[:, mj * 128 : (mj + 1) * 128],
                        xT_sb[:, s],
                        start=True,
                        stop=True,
                    )
                et = hpool.tile([128, BL], fp16, tag=f"h{mj}", name=f"embT{mj}")
                nc.scalar.activation(et[:], eps[:], AF.Relu)
                embT.append(et)

            # ---- setup: A256[m] = 256 * We_m @ embT resident in SBUF (fp16),
            #      with step 0 fused in (gates0 = f(A_raw + bias_d0) from psum,
            #      c1 = i*g, h1 = o*tanh(c1)). ----
            h16 = [None] * K_TILES
            c_sb = [None] * K_TILES
            h8p = [None] * 2
            gact0 = {}
            STEP0_FUNC = {0: AF.Sigmoid, 2: AF.Tanh, 3: AF.Sigmoid}
            # hid-major i,g,o so step-0 cells finish early; f tiles last (f is
            # not needed in step 0, and their A slots hold `we` until the end).
            A_ORDER = [g * 4 + h for h in range(4) for g in (0, 2, 3)] + [4, 5, 6, 7]
            for m in A_ORDER:
                aps = psump.tile([128, BL], fp32, tag="gps", name="aps")
                for k in range(K_TILES):
                    for ncn in range(NC_CH):
                        s = slice(ncn * 512, (ncn + 1) * 512)
                        nc.tensor.matmul(
                            aps[:, s],
                            we_sb[k][:, m * 128 : (m + 1) * 128],
                            embT[k][:, s],
                            start=(k == 0),
                            stop=(k == K_TILES - 1),
                        )
                # stage, then store partition-ROTATED (group q holds logical
                # strip (q-rot)%4) so per-pair replay strips land on distinct
                # PE subarray positions and run concurrently.
                at = apool.tile([128, BL], fp16, tag=f"a{m}", name=f"a{m}")
                ast = workp.tile([128, BL], fp16, tag="z", name="astage")
                nc.scalar.activation(ast[:], aps[:], AF.Copy, scale=SCL * SCL)
                rot = m % 4
                for q in range(4):
                    j = (q - rot) % 4
                    nc.sync.dma_start(
                        at[32 * q : 32 * q + 32, :], ast[32 * j : 32 * j + 32, :]
                    )
                a_sb[m] = at
                gt, hid = divmod(m, 4)
                if gt in STEP0_FUNC:
                    g = workp.tile([128, BL], fp16, tag=f"g{gt}", name=f"g{gt}_0")
                    nc.scalar.activation(
                        g[:], aps[:], STEP0_FUNC[gt], bias=bias_d0[:, m : m + 1]
                    )
                    gact0[(gt, hid)] = g
                if gt == 3:
                    # o-gate staged: finish this hid's step-0 cell update
                    ct = statep.tile([128, BL], fp16, name=f"c{hid}")
                    nc.vector.tensor_tensor(
                        ct[:], gact0[(0, hid)][:], gact0[(2, hid)][:], OP.mult
                    )
                    tt = workp.tile([128, BL], fp16, tag="tt", name="t0")
                    nc.scalar.activation(tt[:], ct[:], AF.Tanh)
                    ht = hpool.tile([128, BL], fp16, tag=f"h{hid}", name=f"h{hid}_0")
                    nc.vector.tensor_tensor(ht[:], gact0[(3, hid)][:], tt[:], OP.mult)
                    h16[hid] = ht
                    c_sb[hid] = ct
                    kp, sl = divmod(hid, 2)
                    if h8p[kp] is None:
                        h8p[kp] = hpool.tile(
                            [128, 2, BL], fp8, tag=f"p{kp}", name=f"p{kp}_0"
                        )
                    nc.vector.scalar_tensor_tensor(
                        h8p[kp][:, sl, :], gact0[(3, hid)][:], SCL, tt[:],
                        OP.mult, OP.mult,
                    )
            h_cur = h16
            h8_cur = h8p

            # ---- steps 1..35 ----
            GATE_FUNC = {0: AF.Sigmoid, 1: AF.Sigmoid, 2: AF.Tanh, 3: AF.Sigmoid}
            inv = 1.0 / (SCL * SCL)

            def y_tile(t_out, h_in):
                """y = W3 @ (16h) / 16 (+ b3) -> out[t_out]."""
                gps = psump.tile([128, BL], fp32, tag="gps", name="yps")
                for k in range(K_TILES):
                    for ncn in range(NC_CH):
                        s = slice(ncn * 512, (ncn + 1) * 512)
                        nc.tensor.matmul(
                            gps[:, s],
                            wg_sb[k][:, 512:640],
                            h_in[k][:, s],
                            start=(k == 0),
                            stop=(k == K_TILES - 1),
                        )
                yr = workp.tile([1, BL], fp16, tag="yrow", bufs=1, name="yrow")
                if y_on_dve:
                    nc.vector.tensor_scalar(
                        yr[:], gps[0:1, :], 1.0, float(b3_val), OP.mult, OP.add
                    )
                else:
                    nc.scalar.activation(
                        yr[:], gps[0:1, :], AF.Copy, bias=float(b3_val), scale=1.0
                    )
                nc.sync.dma_start(out_d[t_out : t_out + 1, :], yr[:])

            for t in range(1, n_steps):
                h_next = [None] * K_TILES
                h8_next = [None] * 2
                for hp in range(2):  # hid pairs (0,1) and (2,3)
                    hids = (2 * hp, 2 * hp + 1)
                    gact = {hid: {} for hid in hids}
                    for gt in range(4):
                        ms = [gt * 4 + hid for hid in hids]
                        gps = {
                            m: psump.tile([128, BL], fp32, tag="gps", name=f"gps{m}")
                            for m in ms
                        }
                        # gate matmuls FIRST (start=True); the A replay comes
                        # last so both psum tiles are live when the strip
                        # packs issue back-to-back (subarray concurrency).
                        for m in ms:
                            hid = m % 4
                            if gt == 2 and g_halfk:
                                gc = slice(hid * 128, (hid + 1) * 128)
                                for ncn in range(NC_CH):
                                    s = slice(ncn * 512, (ncn + 1) * 512)
                                    nc.tensor.matmul(
                                        gps[m][:, s],
                                        w8g_sb[:, :, gc],
                                        h8_cur[0][:, :, s],
                                        start=True,
                                        stop=False,
                                        perf_mode=DR,
                                        skip_group_check=True,
                                    )
                                for k in (2, 3):
                                    for ncn in range(NC_CH):
                                        s = slice(ncn * 512, (ncn + 1) * 512)
                                        nc.tensor.matmul(
                                            gps[m][:, s],
                                            wg_sb[k][:, hid * 128 : (hid + 1) * 128],
                                            h_cur[k][:, s],
                                            start=False,
                                            stop=False,
                                            skip_group_check=True,
                                        )
                            elif gt == 2:
                                for k in range(K_TILES):
                                    for ncn in range(NC_CH):
                                        s = slice(ncn * 512, (ncn + 1) * 512)
                                        nc.tensor.matmul(
                                            gps[m][:, s],
                                            wg_sb[k][:, hid * 128 : (hid + 1) * 128],
                                            h_cur[k][:, s],
                                            start=(k == 0),
                                            stop=False,
                                            skip_group_check=True,
                                        )
                            else:
                                idx = M8.index(m)
                                mc = slice(idx * 128, (idx + 1) * 128)
                                for kp in range(2):
                                    for ncn in range(NC_CH):
                                        s = slice(ncn * 512, (ncn + 1) * 512)
                                        nc.tensor.matmul(
                                            gps[m][:, s],
                                            w8_sb[kp][:, :, mc],
                                            h8_cur[kp][:, :, s],
                                            start=(kp == 0),
                                            stop=False,
                                            perf_mode=DR,
                                            skip_group_check=True,
                                        )
                        # A replay: per chunk, 8 strip MMs (2 m-tiles x 4 row
                        # groups) on distinct (row_grp, col_grp) subarrays.
                        for ncn in range(NC_CH):
                            s = slice(ncn * 512, (ncn + 1) * 512)
                            for m in ms:
                                rot = m % 4
                                for q in range(4):
                                    j = (q - rot) % 4
                                    nc.tensor.matmul(
                                        gps[m][32 * j : 32 * j + 32, s],
                                        ident_sb[
                                            32 * q : 32 * q + 32, 32 * j : 32 * j + 32
                                        ],
                                        a_sb[m][32 * q : 32 * q + 32, s],
                                        start=False,
                                        stop=(q == 3),
                                        tile_position=(32 * q, 32 * j),
                                        skip_group_check=True,
                                    )
                        for m in ms:
                            gt_, hid = divmod(m, 4)
                            g = workp.tile(
                                [128, BL], fp16, tag=f"g{gt}", name=f"g{gt}_{t}"
                            )
                            nc.scalar.activation(
                                g[:], gps[m][:], GATE_FUNC[gt],
                                bias=bias_st[:, m : m + 1], scale=inv,
                            )
                            gact[hid][gt] = g
                    for hid in hids:
                        # cell update for this hid tile (fp16); h8 is written
                        # FIRST on DVE (feeds next step's matmuls); h16 on
                        # GpSimd (only g-gate fp16 k-tiles + y read it).
                        ga = gact[hid]
                        halves = (
                            [slice(0, BL // 2), slice(BL // 2, BL)]
                            if hid in split_hids
                            else [slice(0, BL)]
                        )
                        fc = workp.tile([128, BL], fp16, tag="fcig", bufs=1, name="fc")
                        ig = workp.tile([128, BL], fp16, tag="ig", bufs=1, name="ig")
                        tt = workp.tile([128, BL], fp16, tag="tt", name="tt")
                        ht = hpool.tile(
                            [128, BL], fp16, tag=f"h{hid}", name=f"h{hid}_{t}"
                        )
                        kp, sl = divmod(hid, 2)
                        if h8_next[kp] is None:
                            h8_next[kp] = hpool.tile(
                                [128, 2, BL], fp8, tag=f"p{kp}", name=f"p{kp}_{t}"
                            )
                        for s in halves:
                            nc.vector.tensor_tensor(
                                fc[:, s], ga[1][:, s], c_sb[hid][:, s], OP.mult
                            )
                            nc.vector.tensor_tensor(
                                ig[:, s], ga[0][:, s], ga[2][:, s], OP.mult
                            )
                            nc.vector.tensor_tensor(
                                c_sb[hid][:, s], fc[:, s], ig[:, s], OP.add
                            )
                            nc.scalar.activation(tt[:, s], c_sb[hid][:, s], AF.Tanh)
                            nc.vector.scalar_tensor_tensor(
                                h8_next[kp][:, sl, s], ga[3][:, s], SCL, tt[:, s],
                                OP.mult, OP.mult,
                            )
                            nc.gpsimd.tensor_tensor(
                                ht[:, s], ga[3][:, s], tt[:, s], OP.mult
                            )
                        h_next[hid] = ht
                # y_{t-1} from h_cur (the h this step's matmuls consumed)
                y_tile(t - 1, h_cur)
                h_cur = h_next
                h8_cur = h8_next

            # final output y_{n-1} from the last h
            y_tile(n_steps - 1, h_cur)

    nc.compile()
    return nc


def _prepare_inputs(x, W1, b1, Wih, bih, Whh, bhh, W3, b3):
    """Host-side exact weight folding (fp64) + per-core sharding."""
    wih_col = Wih[:, 511:512].astype(np.float64)  # [2048,1]
    Whh_eff = Whh.astype(np.float64) + wih_col @ W3.astype(np.float64)  # [2048,512]
    bias_steady = (
        bih.astype(np.float64) + bhh.astype(np.float64) + wih_col[:, 0] * float(b3[0])
    )
    # full step-0 bias (applied to the RAW A psum, pre-scale)
    bias_d0 = bih.astype(np.float64) + bhh.astype(np.float64) - 0.8 * wih_col[:, 0]

    WT = (SCL * Whh_eff).T  # [512, 2048] scaled lhsT layout

    # fp8 DoubleRow weights for i/f/o m-tiles: [kp][p][slot][idx*128 + c]
    w8 = np.zeros((2, 128, 2, 12 * 128), np.float32)
    for idx, m in enumerate(M8):
        rows = slice(m * 128, (m + 1) * 128)
        for kp in range(2):
            for sl in range(2):
                k0 = kp * 256 + sl * 128
                w8[kp, :, sl, idx * 128 : (idx + 1) * 128] = WT[k0 : k0 + 128, rows]
    w8 = np.ascontiguousarray(w8).astype(FP8)

    # fp8 DoubleRow g-gate weights, k-pair 0 only (k rows 0-255)
    w8g = np.zeros((128, 2, 512), np.float32)
    for sl in range(2):
        w8g[:, sl, :] = WT[sl * 128 : (sl + 1) * 128, 1024:1536]
    w8g = np.ascontiguousarray(w8g).astype(FP8)

    # fp16 g-gate weights (x256: h16 is unscaled) + W3 col (y = psum as-is)
    wg = np.zeros((HID, 640), np.float64)
    wg[:, :512] = SCL * WT[:, 1024:1536]
    wg[:, 512] = W3[0].astype(np.float64)
    wg = wg.astype(np.float32).astype(FP16)

    we = np.zeros((HID, NG), np.float32)
    we[:EMB, :] = Wih[:, :EMB].T  # row 511 zero (emb row 511 is zero)

    w1t = np.zeros((24, 512), np.float32)
    w1t[:23, :EMB] = W1.T
    w1t[23, :EMB] = b1

    # 32-blocked identity: every aligned 32x32 diagonal block is I32
    ident = (np.arange(128)[:, None] % 32 == np.arange(128)[None, :] % 32).astype(
        np.float32
    )

    bias_st_2d = bias_steady.reshape(M_TILES, 128).T.astype(np.float32)
    bias_d0_2d = bias_d0.reshape(M_TILES, 128).T.astype(np.float32)

    common = {
        "w1t": w1t.astype(FP16),
        "w8": w8,
        "w8g": w8g,
        "wg": wg,
        "we": we.astype(FP16),
        "ident": ident.astype(FP16),
        "bias_st": np.ascontiguousarray(bias_st_2d),
        "bias_d0": np.ascontiguousarray(bias_d0_2d),
    }
    in_maps = []
    for c in range(N_CORES):
        xs = x[c * BL : (c + 1) * BL]  # [BL, 23]
        xT = np.ones((24, BL), np.float32)
        xT[:23, :] = xs.T
        m = dict(common)
        m["xT"] = np.ascontiguousarray(xT).astype(FP16)
        in_maps.append(m)
    return in_maps, float(b3[0])


def kernel(x, W1, b1, Wih, bih, Whh, bhh, W3, b3):
    from concourse.bass_utils import run_bass_kernel_spmd

    in_maps, b3_val = _prepare_inputs(
        np.asarray(x, np.float32),
        np.asarray(W1, np.float32),
        np.asarray(b1, np.float32),
        np.asarray(Wih, np.float32),
        np.asarray(bih, np.float32),
        np.asarray(Whh, np.float32),
        np.asarray(bhh, np.float32),
        np.asarray(W3, np.float32),
        np.asarray(b3, np.float32),
    )
    nc = _build_program(b3_val)
    res = run_bass_kernel_spmd(nc, in_maps, list(range(N_CORES)))
    outs = [np.asarray(res.results[c]["out"]) for c in range(N_CORES)]  # [36, BL] each
    full = np.concatenate(outs, axis=1)  # [36, B]
    return full[:, :, None].astype(np.float32)  # [36, B, 1]


if __name__ == "__main__":
    rng = np.random.default_rng(0)
    ins = {
        "x": rng.standard_normal((B, 23), dtype=np.float32),
        "W1": rng.standard_normal((EMB, 23), dtype=np.float32) / np.sqrt(23),
        "b1": np.zeros(EMB, np.float32),
        "Wih": rng.standard_normal((NG, HID), dtype=np.float32) / np.sqrt(HID),
        "bih": np.zeros(NG, np.float32),
        "Whh": rng.standard_normal((NG, HID), dtype=np.float32) / np.sqrt(HID),
        "bhh": np.zeros(NG, np.float32),
        "W3": rng.standard_normal((1, HID), dtype=np.float32) / np.sqrt(HID),
        "b3": np.zeros(1, np.float32),
    }
    out = kernel(**ins)
    print("kernel output", out.shape, out.dtype, np.abs(out).max())


# revision 20
# speedup vs baseline: 1.1638x; 1.1638x over previous
"""Trainium2 Bass kernel for nn_ModelLSTM (36-step scalar-feedback LSTM).

Model (per reference):
    emb = relu(x @ W1.T + b1)                       # [B, 511], constant across steps
    x0 = -0.8; h0 = c0 = 0
    step t: inp = [emb, xin]                        # [B, 512]
            gates = inp @ Wih.T + bih + h @ Whh.T + bhh
            i,f,g,o = split(gates); c' = sig(f)*c + sig(i)*tanh(g); h' = sig(o)*tanh(c')
            y = h' @ W3.T + b3 ; xin' = y
    output ys = [36, B, 1]

Restructuring (host-side, exact in fp64):
  * xin folds into the recurrent weights: Whh_eff = Whh + Wih[:,511:] @ W3.
  * emb contribution A = Wih[:,:511] @ emb.T is precomputed once on device,
    kept RESIDENT in SBUF (fp16, x256 scale) and replayed into PSUM per step
    via an identity matmul (DVE-add on a couple of low-urgency tiles to
    balance PE vs DVE). Step-0 is fused into the A-precompute pass.

Mixed precision (validated in sim; ~0.016 rel err vs 2e-2 budget):
  * i/f/o gate matmuls: fp8 e4m3 x16-scaled weights AND h, DoubleRow perf
    mode (2 k-tiles per pass -> half the PE time; measured 214ns/MM).
  * g (tanh) gate matmul: fp8 DoubleRow for k rows 0-255, fp16 for rows
    256-511 (tanh slope 1 makes g 4x more error-sensitive; half-K fp8 keeps
    the error at ~sqrt(1/2) of full fp8).
  * Everything else fp16 (gates, h, c, A, emb); PSUM carries a 256x scale
    which ACT's free activation scale undoes.

Sharding: pure data-parallel over batch (16384 -> 8 cores x 2048). Weights
replicated. No collectives.
"""

import sys

for _p in ("/opt/trn_rl_repo",):
    if _p not in sys.path:
        sys.path.insert(0, _p)

import numpy as np
import ml_dtypes

BF16 = ml_dtypes.bfloat16
FP8 = ml_dtypes.float8_e4m3  # TRN variant (inf at 256); values are tiny
FP16 = np.float16

N_CORES = 8
B = 16384
BL = B // N_CORES  # 2048 batch per core
HID = 512
EMB = 511
STEPS = 36
NG = 4 * HID  # 2048 gate rows
M_TILES = 16  # gate row tiles of 128; m = 0..15, gate type = m//4, hid = m%4
K_TILES = 4  # contraction tiles of 128 over HID
NC_CH = BL // 512  # 4 free-dim chunks of 512
M8 = [0, 1, 2, 3, 4, 5, 6, 7, 12, 13, 14, 15]  # i/f/o m-tiles (full fp8 DoubleRow)
SCL = 16.0  # per-operand fp8/fp16 scale; psum carries SCL^2 = 256


def _build_program(
    b3_val: float,
    n_steps: int = STEPS,
    dve_add_ms: tuple = (2, 3, 6, 7, 10, 11),
    g_halfk: bool = True,
    split_hids: tuple = (0, 1),
    y_on_dve: bool = True,
):
    """Build the Bass program.

    dve_add_ms: m-tiles whose A contribution is added on DVE (psum + A -> z)
      instead of a PE identity matmul; balances PE vs DVE. Pick f-tiles of
      hids 2/3 (m=6,7): most slack before their h is consumed.
    g_halfk: g-gate matmul k rows 0-255 in fp8 DoubleRow, 256-511 in fp16.
    split_hids: hids whose cell update runs in column halves, so their h8
      lands early for the next step's first (kp=0) matmuls.
    """
    import concourse.bass as bass
    import concourse.bacc as bacc
    import concourse.tile as tile
    from concourse import mybir

    fp32 = mybir.dt.float32
    fp16 = mybir.dt.float16
    fp8 = mybir.dt.float8e4
    AF = mybir.ActivationFunctionType
    OP = mybir.AluOpType
    DR = mybir.MatmulPerfMode.DoubleRow

    nc = bacc.Bacc(
        "TRN2",
        target_bir_lowering=False,
        debug=False,
        num_devices=N_CORES,
    )

    # ---- DRAM I/O (per-core shapes) ----
    xT_d = nc.dram_tensor("xT", [24, BL], fp16, kind="ExternalInput")
    w1t_d = nc.dram_tensor("w1t", [24, 512], fp16, kind="ExternalInput")
    w8_d = nc.dram_tensor("w8", [2, 128, 2, 12 * 128], fp8, kind="ExternalInput")
    w8g_d = nc.dram_tensor("w8g", [128, 2, 512], fp8, kind="ExternalInput")
    wg_d = nc.dram_tensor("wg", [HID, 640], fp16, kind="ExternalInput")
    we_d = nc.dram_tensor("we", [HID, NG], fp16, kind="ExternalInput")
    ident_d = nc.dram_tensor("ident", [128, 128], fp16, kind="ExternalInput")
    bias_st_d = nc.dram_tensor("bias_st", [128, M_TILES], fp32, kind="ExternalInput")
    bias_d0_d = nc.dram_tensor("bias_d0", [128, M_TILES], fp32, kind="ExternalInput")
    out_d = nc.dram_tensor("out", [STEPS, BL], fp16, kind="ExternalOutput")

    with tile.TileContext(nc) as tc:
        with (
            tc.tile_pool(name="const", bufs=1) as constp,
            tc.tile_pool(name="apool", bufs=1) as apool,
            tc.tile_pool(name="state", bufs=1) as statep,
            tc.tile_pool(name="hpool", bufs=2) as hpool,
            tc.tile_pool(name="work", bufs=2) as workp,
            tc.tile_pool(name="psum", bufs=2, space=bass.MemorySpace.PSUM) as psump,
        ):
            # ---- load constants ----
            xT_sb = workp.tile([24, BL], fp16, tag="xT", bufs=1, name="xT_sb")
            nc.sync.dma_start(xT_sb[:], xT_d[:])
            w1t_sb = workp.tile([24, 512], fp16, tag="tt", name="w1t_sb")
            nc.sync.dma_start(w1t_sb[:], w1t_d[:])
            w8_sb = []
            for kp in range(2):
                wt = constp.tile([128, 2, 12 * 128], fp8, name=f"w8_{kp}")
                nc.sync.dma_start(wt[:], w8_d[kp])
                w8_sb.append(wt)
            w8g_sb = constp.tile([128, 2, 512], fp8, name="w8g")
            nc.sync.dma_start(w8g_sb[:], w8g_d[:])
            wg_sb = []
            for k in range(K_TILES):
                wt = constp.tile([128, 640], fp16, name=f"wg{k}")
                nc.sync.dma_start(wt[:], wg_d[k * 128 : (k + 1) * 128, :])
                wg_sb.append(wt)
            ident_sb = constp.tile([128, 128], fp16, name="ident")
            nc.sync.dma_start(ident_sb[:], ident_d[:])
            bias_st = constp.tile([128, M_TILES], fp32, name="bias_st")
            nc.sync.dma_start(bias_st[:], bias_st_d[:])
            bias_d0 = constp.tile([128, M_TILES], fp32, name="bias_d0")
            nc.sync.dma_start(bias_d0[:], bias_d0_d[:])

            # we k-tiles parked in the f-gate A slots (a4..a7) - those A tiles
            # are produced LAST in A_ORDER, and the a_stage hop decouples the
            # writeback so the slots recycle without deadlock.
            a_sb = [None] * M_TILES
            we_sb = []
            for k in range(K_TILES):
                wt = apool.tile([128, BL], fp16, tag=f"a{4 + k}", name=f"we{k}")
                nc.sync.dma_start(wt[:], we_d[k * 128 : (k + 1) * 128, :])
                we_sb.append(wt)

            # ---- setup: emb = relu(x @ W1.T + b1) (transposed, fp16) ----
            # emb k-tiles borrow the h16 tags (free during setup; step-0 h16
            # writes land in buf 1 of those tags).
            embT = []
            for mj in range(4):
                eps = psump.tile([128, BL], fp32, tag="gps", name="eps")
                for ncn in range(NC_CH):
                    s = slice(ncn * 512, (ncn + 1) * 512)
                    nc.tensor.matmul(
                        eps[:, s],
                        w1t_sb[:, mj * 128 : (mj + 1) * 128],
                        xT_sb[:, s],
                        start=True,
                        stop=True,
                    )
                et = hpool.tile([128, BL], fp16, tag=f"h{mj}", name=f"embT{mj}")
                nc.scalar.activation(et[:], eps[:], AF.Relu)
                embT.append(et)

            # ---- setup: A256[m] = 256 * We_m @ embT resident in SBUF (fp16),
            #      with step 0 fused in (gates0 = f(A_raw + bias_d0) from psum,
            #      c1 = i*g, h1 = o*tanh(c1)). ----
            h16 = [None] * K_TILES
            c_sb = [None] * K_TILES
            h8p = [None] * 2
            gact0 = {}
            STEP0_FUNC = {0: AF.Sigmoid, 2: AF.Tanh, 3: AF.Sigmoid}
            # hid-major i,g,o so step-0 cells finish early; f tiles last (f is
            # not needed in step 0, and their A slots hold `we` until the end).
            A_ORDER = [g * 4 + h for h in range(4) for g in (0, 2, 3)] + [4, 5, 6, 7]
            for m in A_ORDER:
                aps = psump.tile([128, BL], fp32, tag="gps", name="aps")
                for k in range(K_TILES):
                    for ncn in range(NC_CH):
                        s = slice(ncn * 512, (ncn + 1) * 512)
                        nc.tensor.matmul(
                            aps[:, s],
                            we_sb[k][:, m * 128 : (m + 1) * 128],
                            embT[k][:, s],
                            start=(k == 0),
                            stop=(k == K_TILES - 1),
                        )
                at = apool.tile([128, BL], fp16, tag=f"a{m}", name=f"a{m}")
                if 4 <= m <= 7:
                    # f-tile slots hold `we` until the end; stage + DMA hop
                    ast = workp.tile([128, BL], fp16, tag="z", name="astage")
                    nc.scalar.activation(ast[:], aps[:], AF.Copy, scale=SCL * SCL)
                    nc.sync.dma_start(at[:], ast[:])
                else:
                    nc.scalar.activation(at[:], aps[:], AF.Copy, scale=SCL * SCL)
                a_sb[m] = at
                gt, hid = divmod(m, 4)
                if gt in STEP0_FUNC:
                    g = workp.tile([128, BL], fp16, tag=f"g{gt}", name=f"g{gt}_0")
                    nc.scalar.activation(
                        g[:], aps[:], STEP0_FUNC[gt], bias=bias_d0[:, m : m + 1]
                    )
                    gact0[(gt, hid)] = g
                if gt == 3:
                    # o-gate staged: finish this hid's step-0 cell update
                    ct = statep.tile([128, BL], fp16, name=f"c{hid}")
                    nc.vector.tensor_tensor(
                        ct[:], gact0[(0, hid)][:], gact0[(2, hid)][:], OP.mult
                    )
                    tt = workp.tile([128, BL], fp16, tag="tt", name="t0")
                    nc.scalar.activation(tt[:], ct[:], AF.Tanh)
                    ht = hpool.tile([128, BL], fp16, tag=f"h{hid}", name=f"h{hid}_0")
                    nc.vector.tensor_tensor(ht[:], gact0[(3, hid)][:], tt[:], OP.mult)
                    h16[hid] = ht
                    c_sb[hid] = ct
                    kp, sl = divmod(hid, 2)
                    if h8p[kp] is None:
                        h8p[kp] = hpool.tile(
                            [128, 2, BL], fp8, tag=f"p{kp}", name=f"p{kp}_0"
                        )
                    nc.vector.scalar_tensor_tensor(
                        h8p[kp][:, sl, :], gact0[(3, hid)][:], SCL, tt[:],
                        OP.mult, OP.mult,
                    )
            h_cur = h16
            h8_cur = h8p

            # ---- steps 1..35 ----
            GATE_FUNC = {0: AF.Sigmoid, 1: AF.Sigmoid, 2: AF.Tanh, 3: AF.Sigmoid}
            inv = 1.0 / (SCL * SCL)

            def y_tile(t_out, h_in):
                """y = W3 @ (16h) / 16 (+ b3) -> out[t_out]."""
                gps = psump.tile([128, BL], fp32, tag="gps", name="yps")
                for k in range(K_TILES):
                    for ncn in range(NC_CH):
                        s = slice(ncn * 512, (ncn + 1) * 512)
                        nc.tensor.matmul(
                            gps[:, s],
                            wg_sb[k][:, 512:640],
                            h_in[k][:, s],
                            start=(k == 0),
                            stop=(k == K_TILES - 1),
                        )
                yr = workp.tile([1, BL], fp16, tag="yrow", bufs=1, name="yrow")
                if y_on_dve:
                    nc.vector.tensor_scalar(
                        yr[:], gps[0:1, :], 1.0, float(b3_val), OP.mult, OP.add
                    )
                else:
                    nc.scalar.activation(
                        yr[:], gps[0:1, :], AF.Copy, bias=float(b3_val), scale=1.0
                    )
                nc.sync.dma_start(out_d[t_out : t_out + 1, :], yr[:])

            for t in range(1, n_steps):
                h_next = [None] * K_TILES
                h8_next = [None] * 2
                for hid in range(K_TILES):
                    gact = {}
                    for gt in range(4):
                        m = gt * 4 + hid
                        use_dve = m in dve_add_ms
                        gps = psump.tile([128, BL], fp32, tag="gps", name="gps")
                        if not use_dve:
                            for ncn in range(NC_CH):
                                s = slice(ncn * 512, (ncn + 1) * 512)
                                nc.tensor.matmul(
                                    gps[:, s], ident_sb[:], a_sb[m][:, s],
                                    start=True, stop=False,
                                )
                        if gt == 2 and g_halfk:
                            # g: fp8 DoubleRow for k 0-255, fp16 for k 256-511
                            gc = slice(hid * 128, (hid + 1) * 128)
                            for ncn in range(NC_CH):
                                s = slice(ncn * 512, (ncn + 1) * 512)
                                nc.tensor.matmul(
                                    gps[:, s],
                                    w8g_sb[:, :, gc],
                                    h8_cur[0][:, :, s],
                                    start=use_dve,
                                    stop=False,
                                    perf_mode=DR,
                                )
                            for k in (2, 3):
                                for ncn in range(NC_CH):
                                    s = slice(ncn * 512, (ncn + 1) * 512)
                                    nc.tensor.matmul(
                                        gps[:, s],
                                        wg_sb[k][:, hid * 128 : (hid + 1) * 128],
                                        h_cur[k][:, s],
                                        start=False,
                                        stop=(k == K_TILES - 1),
                                    )
                        elif gt == 2:
                            for k in range(K_TILES):
                                for ncn in range(NC_CH):
                                    s = slice(ncn * 512, (ncn + 1) * 512)
                                    nc.tensor.matmul(
                                        gps[:, s],
                                        wg_sb[k][:, hid * 128 : (hid + 1) * 128],
                                        h_cur[k][:, s],
                                        start=(k == 0 and use_dve),
                                        stop=(k == K_TILES - 1),
                                    )
                        else:
                            # i/f/o: fp8 DoubleRow over 2 k-pairs
                            idx = M8.index(m)
                            mc = slice(idx * 128, (idx + 1) * 128)
                            for kp in range(2):
                                for ncn in range(NC_CH):
                                    s = slice(ncn * 512, (ncn + 1) * 512)
                                    nc.tensor.matmul(
                                        gps[:, s],
                                        w8_sb[kp][:, :, mc],
                                        h8_cur[kp][:, :, s],
                                        start=(kp == 0 and use_dve),
                                        stop=(kp == 1),
                                        perf_mode=DR,
                                    )
                        g = workp.tile([128, BL], fp16, tag=f"g{gt}", name=f"g{gt}_{t}")
                        if use_dve:
                            z = workp.tile([128, BL], fp16, tag="z", name="z")
                            nc.vector.tensor_tensor(z[:], gps[:], a_sb[m][:], OP.add)
                            nc.scalar.activation(
                                g[:], z[:], GATE_FUNC[gt],
                                bias=bias_st[:, m : m + 1], scale=inv,
                            )
                        else:
                            nc.scalar.activation(
                                g[:], gps[:], GATE_FUNC[gt],
                                bias=bias_st[:, m : m + 1], scale=inv,
                            )
                        gact[gt] = g
                    # cell update for this hid tile (all fp16 on DVE); h8 is
                    # written FIRST (it feeds the next step's matmuls), h16
                    # second (g-gate fp16 k-tiles + y only).
                    halves = (
                        [slice(0, BL // 2), slice(BL // 2, BL)]
                        if hid in split_hids
                        else [slice(0, BL)]
                    )
                    fc = workp.tile([128, BL], fp16, tag="fcig", bufs=1, name="fc")
                    ig = workp.tile([128, BL], fp16, tag="ig", bufs=1, name="ig")
                    tt = workp.tile([128, BL], fp16, tag="tt", name="tt")
                    ht = hpool.tile([128, BL], fp16, tag=f"h{hid}", name=f"h{hid}_{t}")
                    kp, sl = divmod(hid, 2)
                    if h8_next[kp] is None:
                        h8_next[kp] = hpool.tile(
                            [128, 2, BL], fp8, tag=f"p{kp}", name=f"p{kp}_{t}"
                        )
                    for s in halves:
                        nc.vector.tensor_tensor(
                            fc[:, s], gact[1][:, s], c_sb[hid][:, s], OP.mult
                        )
                        nc.vector.tensor_tensor(
                            ig[:, s], gact[0][:, s], gact[2][:, s], OP.mult
                        )
                        nc.vector.tensor_tensor(
                            c_sb[hid][:, s], fc[:, s], ig[:, s], OP.add
                        )
                        nc.scalar.activation(tt[:, s], c_sb[hid][:, s], AF.Tanh)
                        nc.vector.scalar_tensor_tensor(
                            h8_next[kp][:, sl, s], gact[3][:, s], SCL, tt[:, s],
                            OP.mult, OP.mult,
                        )
                        nc.gpsimd.tensor_tensor(
                            ht[:, s], gact[3][:, s], tt[:, s], OP.mult
                        )
                    h_next[hid] = ht
                # y_{t-1} from h_cur (the h this step's matmuls consumed)
                y_tile(t - 1, h_cur)
                h_cur = h_next
                h8_cur = h8_next

            # final output y_{n-1} from the last h
            y_tile(n_steps - 1, h_cur)

    nc.compile()
    return nc


def _prepare_inputs(x, W1, b1, Wih, bih, Whh, bhh, W3, b3):
    """Host-side exact weight folding (fp64) + per-core sharding."""
    wih_col = Wih[:, 511:512].astype(np.float64)  # [2048,1]
    Whh_eff = Whh.astype(np.float64) + wih_col @ W3.astype(np.float64)  # [2048,512]
    bias_steady = (
        bih.astype(np.float64) + bhh.astype(np.float64) + wih_col[:, 0] * float(b3[0])
    )
    # full step-0 bias (applied to the RAW A psum, pre-scale)
    bias_d0 = bih.astype(np.float64) + bhh.astype(np.float64) - 0.8 * wih_col[:, 0]

    WT = (SCL * Whh_eff).T  # [512, 2048] scaled lhsT layout

    # fp8 DoubleRow weights for i/f/o m-tiles: [kp][p][slot][idx*128 + c]
    w8 = np.zeros((2, 128, 2, 12 * 128), np.float32)
    for idx, m in enumerate(M8):
        rows = slice(m * 128, (m + 1) * 128)
        for kp in range(2):
            for sl in range(2):
                k0 = kp * 256 + sl * 128
                w8[kp, :, sl, idx * 128 : (idx + 1) * 128] = WT[k0 : k0 + 128, rows]
    w8 = np.ascontiguousarray(w8).astype(FP8)

    # fp8 DoubleRow g-gate weights, k-pair 0 only (k rows 0-255)
    w8g = np.zeros((128, 2, 512), np.float32)
    for sl in range(2):
        w8g[:, sl, :] = WT[sl * 128 : (sl + 1) * 128, 1024:1536]
    w8g = np.ascontiguousarray(w8g).astype(FP8)

    # fp16 g-gate weights (x256: h16 is unscaled) + W3 col (y = psum as-is)
    wg = np.zeros((HID, 640), np.float64)
    wg[:, :512] = SCL * WT[:, 1024:1536]
    wg[:, 512] = W3[0].astype(np.float64)
    wg = wg.astype(np.float32).astype(FP16)

    we = np.zeros((HID, NG), np.float32)
    we[:EMB, :] = Wih[:, :EMB].T  # row 511 zero (emb row 511 is zero)

    w1t = np.zeros((24, 512), np.float32)
    w1t[:23, :EMB] = W1.T
    w1t[23, :EMB] = b1

    ident = np.eye(128, dtype=np.float32)

    bias_st_2d = bias_steady.reshape(M_TILES, 128).T.astype(np.float32)
    bias_d0_2d = bias_d0.reshape(M_TILES, 128).T.astype(np.float32)

    common = {
        "w1t": w1t.astype(FP16),
        "w8": w8,
        "w8g": w8g,
        "wg": wg,
        "we": we.astype(FP16),
        "ident": ident.astype(FP16),
        "bias_st": np.ascontiguousarray(bias_st_2d),
        "bias_d0": np.ascontiguousarray(bias_d0_2d),
    }
    in_maps = []
    for c in range(N_CORES):
        xs = x[c * BL : (c + 1) * BL]  # [BL, 23]
        xT = np.ones((24, BL), np.float32)
        xT[:23, :] = xs.T
        m = dict(common)
        m["xT"] = np.ascontiguousarray(xT).astype(FP16)
        in_maps.append(m)
    return in_maps, float(b3[0])


def kernel(x, W1, b1, Wih, bih, Whh, bhh, W3, b3):
    from concourse.bass_utils import run_bass_kernel_spmd

    in_maps, b3_val = _prepare_inputs(
        np.asarray(x, np.float32),
        np.asarray(W1, np.float32),
        np.asarray(b1, np.float32),
        np.asarray(Wih, np.float32),
        np.asarray(bih, np.float32),
        np.asarray(Whh, np.float32),
        np.asarray(bhh, np.float32),
        np.asarray(W3, np.float32),
        np.asarray(b3, np.float32),
    )
    nc = _build_program(b3_val)
    res = run_bass_kernel_spmd(nc, in_maps, list(range(N_CORES)))
    outs = [np.asarray(res.results[c]["out"]) for c in range(N_CORES)]  # [36, BL] each
    full = np.concatenate(outs, axis=1)  # [36, B]
    return full[:, :, None].astype(np.float32)  # [36, B, 1]


if __name__ == "__main__":
    rng = np.random.default_rng(0)
    ins = {
        "x": rng.standard_normal((B, 23), dtype=np.float32),
        "W1": rng.standard_normal((EMB, 23), dtype=np.float32) / np.sqrt(23),
        "b1": np.zeros(EMB, np.float32),
        "Wih": rng.standard_normal((NG, HID), dtype=np.float32) / np.sqrt(HID),
        "bih": np.zeros(NG, np.float32),
        "Whh": rng.standard_normal((NG, HID), dtype=np.float32) / np.sqrt(HID),
        "bhh": np.zeros(NG, np.float32),
        "W3": rng.standard_normal((1, HID), dtype=np.float32) / np.sqrt(HID),
        "b3": np.zeros(1, np.float32),
    }
    out = kernel(**ins)
    print("kernel output", out.shape, out.dtype, np.abs(out).max())


# revision 21
# speedup vs baseline: 1.2635x; 1.0857x over previous
"""Trainium2 Bass kernel for nn_ModelLSTM (36-step scalar-feedback LSTM).

Model (per reference):
    emb = relu(x @ W1.T + b1)                       # [B, 511], constant across steps
    x0 = -0.8; h0 = c0 = 0
    step t: inp = [emb, xin]                        # [B, 512]
            gates = inp @ Wih.T + bih + h @ Whh.T + bhh
            i,f,g,o = split(gates); c' = sig(f)*c + sig(i)*tanh(g); h' = sig(o)*tanh(c')
            y = h' @ W3.T + b3 ; xin' = y
    output ys = [36, B, 1]

Restructuring (host-side, exact in fp64):
  * xin folds into the recurrent weights: Whh_eff = Whh + Wih[:,511:] @ W3.
  * emb contribution A = Wih[:,:511] @ emb.T is precomputed once on device,
    kept RESIDENT in SBUF (fp16, x256 scale) and replayed into PSUM per step
    via an identity matmul (DVE-add on a couple of low-urgency tiles to
    balance PE vs DVE). Step-0 is fused into the A-precompute pass.

Mixed precision (validated in sim; ~0.016 rel err vs 2e-2 budget):
  * i/f/o gate matmuls: fp8 e4m3 x16-scaled weights AND h, DoubleRow perf
    mode (2 k-tiles per pass -> half the PE time; measured 214ns/MM).
  * g (tanh) gate matmul: fp8 DoubleRow for k rows 0-255, fp16 for rows
    256-511 (tanh slope 1 makes g 4x more error-sensitive; half-K fp8 keeps
    the error at ~sqrt(1/2) of full fp8).
  * Everything else fp16 (gates, h, c, A, emb); PSUM carries a 256x scale
    which ACT's free activation scale undoes.

Sharding: pure data-parallel over batch (16384 -> 8 cores x 2048). Weights
replicated. No collectives.
"""

import sys

for _p in ("/opt/trn_rl_repo",):
    if _p not in sys.path:
        sys.path.insert(0, _p)

import numpy as np
import ml_dtypes

BF16 = ml_dtypes.bfloat16
FP8 = ml_dtypes.float8_e4m3  # TRN variant (inf at 256); values are tiny
FP16 = np.float16

N_CORES = 8
B = 16384
BL = B // N_CORES  # 2048 batch per core
HID = 512
EMB = 511
STEPS = 36
NG = 4 * HID  # 2048 gate rows
M_TILES = 16  # gate row tiles of 128; m = 0..15, gate type = m//4, hid = m%4
K_TILES = 4  # contraction tiles of 128 over HID
NC_CH = BL // 512  # 4 free-dim chunks of 512
M8 = [0, 1, 2, 3, 4, 5, 6, 7, 12, 13, 14, 15]  # i/f/o m-tiles (full fp8 DoubleRow)
SCL = 16.0  # per-operand fp8/fp16 scale; psum carries SCL^2 = 256


def _build_program(
    b3_val: float,
    n_steps: int = STEPS,
    dve_add_ms: tuple = (6, 7),
    g_halfk: bool = True,
    split_hids: tuple = (0, 1),
    y_on_dve: bool = True,
):
    """Build the Bass program.

    dve_add_ms: m-tiles whose A contribution is added on DVE (psum + A -> z)
      instead of a PE identity matmul; balances PE vs DVE. Pick f-tiles of
      hids 2/3 (m=6,7): most slack before their h is consumed.
    g_halfk: g-gate matmul k rows 0-255 in fp8 DoubleRow, 256-511 in fp16.
    split_hids: hids whose cell update runs in column halves, so their h8
      lands early for the next step's first (kp=0) matmuls.
    """
    import concourse.bass as bass
    import concourse.bacc as bacc
    import concourse.tile as tile
    from concourse import mybir

    fp32 = mybir.dt.float32
    fp16 = mybir.dt.float16
    fp8 = mybir.dt.float8e4
    AF = mybir.ActivationFunctionType
    OP = mybir.AluOpType
    DR = mybir.MatmulPerfMode.DoubleRow

    nc = bacc.Bacc(
        "TRN2",
        target_bir_lowering=False,
        debug=False,
        num_devices=N_CORES,
    )

    # ---- DRAM I/O (per-core shapes) ----
    xT_d = nc.dram_tensor("xT", [24, BL], fp16, kind="ExternalInput")
    w1t_d = nc.dram_tensor("w1t", [24, 512], fp16, kind="ExternalInput")
    w8_d = nc.dram_tensor("w8", [2, 128, 2, 12 * 128], fp8, kind="ExternalInput")
    w8g_d = nc.dram_tensor("w8g", [128, 2, 512], fp8, kind="ExternalInput")
    wg_d = nc.dram_tensor("wg", [HID, 640], fp16, kind="ExternalInput")
    we_d = nc.dram_tensor("we", [HID, NG], fp16, kind="ExternalInput")
    ident_d = nc.dram_tensor("ident", [128, 128], fp16, kind="ExternalInput")
    bias_st_d = nc.dram_tensor("bias_st", [128, M_TILES], fp32, kind="ExternalInput")
    bias_d0_d = nc.dram_tensor("bias_d0", [128, M_TILES], fp32, kind="ExternalInput")
    out_d = nc.dram_tensor("out", [STEPS, BL], fp16, kind="ExternalOutput")

    with tile.TileContext(nc) as tc:
        with (
            tc.tile_pool(name="const", bufs=1) as constp,
            tc.tile_pool(name="apool", bufs=1) as apool,
            tc.tile_pool(name="state", bufs=1) as statep,
            tc.tile_pool(name="hpool", bufs=2) as hpool,
            tc.tile_pool(name="work", bufs=2) as workp,
            tc.tile_pool(name="psum", bufs=2, space=bass.MemorySpace.PSUM) as psump,
        ):
            # ---- load constants ----
            xT_sb = workp.tile([24, BL], fp16, tag="xT", bufs=1, name="xT_sb")
            nc.sync.dma_start(xT_sb[:], xT_d[:])
            w1t_sb = workp.tile([24, 512], fp16, tag="tt", name="w1t_sb")
            nc.sync.dma_start(w1t_sb[:], w1t_d[:])
            w8_sb = []
            for kp in range(2):
                wt = constp.tile([128, 2, 12 * 128], fp8, name=f"w8_{kp}")
                nc.sync.dma_start(wt[:], w8_d[kp])
                w8_sb.append(wt)
            w8g_sb = constp.tile([128, 2, 512], fp8, name="w8g")
            nc.sync.dma_start(w8g_sb[:], w8g_d[:])
            wg_sb = []
            for k in range(K_TILES):
                wt = constp.tile([128, 640], fp16, name=f"wg{k}")
                nc.sync.dma_start(wt[:], wg_d[k * 128 : (k + 1) * 128, :])
                wg_sb.append(wt)
            ident_sb = constp.tile([128, 128], fp16, name="ident")
            nc.sync.dma_start(ident_sb[:], ident_d[:])
            bias_st = constp.tile([128, M_TILES], fp32, name="bias_st")
            nc.sync.dma_start(bias_st[:], bias_st_d[:])
            bias_d0 = constp.tile([128, M_TILES], fp32, name="bias_d0")
            nc.sync.dma_start(bias_d0[:], bias_d0_d[:])

            # we k-tiles parked in the f-gate A slots (a4..a7) - those A tiles
            # are produced LAST in A_ORDER, and the a_stage hop decouples the
            # writeback so the slots recycle without deadlock.
            a_sb = [None] * M_TILES
            we_sb = []
            for k in range(K_TILES):
                wt = apool.tile([128, BL], fp16, tag=f"a{4 + k}", name=f"we{k}")
                nc.sync.dma_start(wt[:], we_d[k * 128 : (k + 1) * 128, :])
                we_sb.append(wt)

            # ---- setup: emb = relu(x @ W1.T + b1) (transposed, fp16) ----
            # emb k-tiles borrow the h16 tags (free during setup; step-0 h16
            # writes land in buf 1 of those tags).
            embT = []
            for mj in range(4):
                eps = psump.tile([128, BL], fp32, tag="gps", name="eps")
                for ncn in range(NC_CH):
                    s = slice(ncn * 512, (ncn + 1) * 512)
                    nc.tensor.matmul(
                        eps[:, s],
                        w1t_sb[:, mj * 128 : (mj + 1) * 128],
                        xT_sb[:, s],
                        start=True,
                        stop=True,
                    )
                et = hpool.tile([128, BL], fp16, tag=f"h{mj}", name=f"embT{mj}")
                nc.scalar.activation(et[:], eps[:], AF.Relu)
                embT.append(et)

            # ---- setup: A256[m] = 256 * We_m @ embT resident in SBUF (fp16),
            #      with step 0 fused in (gates0 = f(A_raw + bias_d0) from psum,
            #      c1 = i*g, h1 = o*tanh(c1)). ----
            h16 = [None] * K_TILES
            c_sb = [None] * K_TILES
            h8p = [None] * 2
            gact0 = {}
            STEP0_FUNC = {0: AF.Sigmoid, 2: AF.Tanh, 3: AF.Sigmoid}
            # hid-major i,g,o so step-0 cells finish early; f tiles last (f is
            # not needed in step 0, and their A slots hold `we` until the end).
            A_ORDER = [g * 4 + h for h in range(4) for g in (0, 2, 3)] + [4, 5, 6, 7]
            for m in A_ORDER:
                aps = psump.tile([128, BL], fp32, tag="gps", name="aps")
                for k in range(K_TILES):
                    for ncn in range(NC_CH):
                        s = slice(ncn * 512, (ncn + 1) * 512)
                        nc.tensor.matmul(
                            aps[:, s],
                            we_sb[k][:, m * 128 : (m + 1) * 128],
                            embT[k][:, s],
                            start=(k == 0),
                            stop=(k == K_TILES - 1),
                        )
                at = apool.tile([128, BL], fp16, tag=f"a{m}", name=f"a{m}")
                if 4 <= m <= 7:
                    # f-tile slots hold `we` until the end; stage + DMA hop
                    ast = workp.tile([128, BL], fp16, tag="z", name="astage")
                    nc.scalar.activation(ast[:], aps[:], AF.Copy, scale=SCL * SCL)
                    nc.sync.dma_start(at[:], ast[:])
                else:
                    nc.scalar.activation(at[:], aps[:], AF.Copy, scale=SCL * SCL)
                a_sb[m] = at
                gt, hid = divmod(m, 4)
                if gt in STEP0_FUNC:
                    g = workp.tile([128, BL], fp16, tag=f"g{gt}", name=f"g{gt}_0")
                    nc.scalar.activation(
                        g[:], aps[:], STEP0_FUNC[gt], bias=bias_d0[:, m : m + 1]
                    )
                    gact0[(gt, hid)] = g
                if gt == 3:
                    # o-gate staged: finish this hid's step-0 cell update
                    ct = statep.tile([128, BL], fp16, name=f"c{hid}")
                    nc.vector.tensor_tensor(
                        ct[:], gact0[(0, hid)][:], gact0[(2, hid)][:], OP.mult
                    )
                    tt = workp.tile([128, BL], fp16, tag="tt", name="t0")
                    nc.scalar.activation(tt[:], ct[:], AF.Tanh)
                    ht = hpool.tile([128, BL], fp16, tag=f"h{hid}", name=f"h{hid}_0")
                    nc.vector.scalar_tensor_tensor(
                        ht[:], gact0[(3, hid)][:], SCL, tt[:], OP.mult, OP.mult
                    )
                    h16[hid] = ht
                    c_sb[hid] = ct
                    kp, sl = divmod(hid, 2)
                    if h8p[kp] is None:
                        h8p[kp] = hpool.tile(
                            [128, 2, BL], fp8, tag=f"p{kp}", name=f"p{kp}_0"
                        )
                    nc.vector.tensor_copy(h8p[kp][:, sl, :], ht[:])
            h_cur = h16
            h8_cur = h8p

            # ---- steps 1..35 ----
            GATE_FUNC = {0: AF.Sigmoid, 1: AF.Sigmoid, 2: AF.Tanh, 3: AF.Sigmoid}
            inv = 1.0 / (SCL * SCL)

            def y_tile(t_out, h_in):
                """y = W3 @ (16h) / 16 (+ b3) -> out[t_out]."""
                gps = psump.tile([128, BL], fp32, tag="gps", name="yps")
                for k in range(K_TILES):
                    for ncn in range(NC_CH):
                        s = slice(ncn * 512, (ncn + 1) * 512)
                        nc.tensor.matmul(
                            gps[:, s],
                            wg_sb[k][:, 512:640],
                            h_in[k][:, s],
                            start=(k == 0),
                            stop=(k == K_TILES - 1),
                        )
                yr = workp.tile([1, BL], fp16, tag="yrow", bufs=1, name="yrow")
                if y_on_dve:
                    nc.vector.tensor_scalar(
                        yr[:], gps[0:1, :], 1.0 / SCL, float(b3_val), OP.mult, OP.add
                    )
                else:
                    nc.scalar.activation(
                        yr[:], gps[0:1, :], AF.Copy, bias=float(b3_val), scale=1.0 / SCL
                    )
                nc.sync.dma_start(out_d[t_out : t_out + 1, :], yr[:])

            for t in range(1, n_steps):
                h_next = [None] * K_TILES
                h8_next = [None] * 2
                for hid in range(K_TILES):
                    gact = {}
                    for gt in range(4):
                        m = gt * 4 + hid
                        use_dve = m in dve_add_ms
                        gps = psump.tile([128, BL], fp32, tag="gps", name="gps")
                        if not use_dve:
                            for ncn in range(NC_CH):
                                s = slice(ncn * 512, (ncn + 1) * 512)
                                nc.tensor.matmul(
                                    gps[:, s], ident_sb[:], a_sb[m][:, s],
                                    start=True, stop=False,
                                )
                        if gt == 2 and g_halfk:
                            # g: fp8 DoubleRow for k 0-255, fp16 for k 256-511
                            gc = slice(hid * 128, (hid + 1) * 128)
                            for ncn in range(NC_CH):
                                s = slice(ncn * 512, (ncn + 1) * 512)
                                nc.tensor.matmul(
                                    gps[:, s],
                                    w8g_sb[:, :, gc],
                                    h8_cur[0][:, :, s],
                                    start=use_dve,
                                    stop=False,
                                    perf_mode=DR,
                                )
                            for k in (2, 3):
                                for ncn in range(NC_CH):
                                    s = slice(ncn * 512, (ncn + 1) * 512)
                                    nc.tensor.matmul(
                                        gps[:, s],
                                        wg_sb[k][:, hid * 128 : (hid + 1) * 128],
                                        h_cur[k][:, s],
                                        start=False,
                                        stop=(k == K_TILES - 1),
                                    )
                        elif gt == 2:
                            for k in range(K_TILES):
                                for ncn in range(NC_CH):
                                    s = slice(ncn * 512, (ncn + 1) * 512)
                                    nc.tensor.matmul(
                                        gps[:, s],
                                        wg_sb[k][:, hid * 128 : (hid + 1) * 128],
                                        h_cur[k][:, s],
                                        start=(k == 0 and use_dve),
                                        stop=(k == K_TILES - 1),
                                    )
                        else:
                            # i/f/o: fp8 DoubleRow over 2 k-pairs
                            idx = M8.index(m)
                            mc = slice(idx * 128, (idx + 1) * 128)
                            for kp in range(2):
                                for ncn in range(NC_CH):
                                    s = slice(ncn * 512, (ncn + 1) * 512)
                                    nc.tensor.matmul(
                                        gps[:, s],
                                        w8_sb[kp][:, :, mc],
                                        h8_cur[kp][:, :, s],
                                        start=(kp == 0 and use_dve),
                                        stop=(kp == 1),
                                        perf_mode=DR,
                                    )
                        g = workp.tile([128, BL], fp16, tag=f"g{gt}", name=f"g{gt}_{t}")
                        if use_dve:
                            z = workp.tile([128, BL], fp16, tag="z", name="z")
                            nc.vector.tensor_tensor(z[:], gps[:], a_sb[m][:], OP.add)
                            nc.scalar.activation(
                                g[:], z[:], GATE_FUNC[gt],
                                bias=bias_st[:, m : m + 1], scale=inv,
                            )
                        else:
                            nc.scalar.activation(
                                g[:], gps[:], GATE_FUNC[gt],
                                bias=bias_st[:, m : m + 1], scale=inv,
                            )
                        gact[gt] = g
                    # cell update for this hid tile (all fp16 on DVE); h8 is
                    # written FIRST (it feeds the next step's matmuls), h16
                    # second (g-gate fp16 k-tiles + y only).
                    halves = (
                        [slice(0, BL // 2), slice(BL // 2, BL)]
                        if hid in split_hids
                        else [slice(0, BL)]
                    )
                    fc = workp.tile([128, BL], fp16, tag="fcig", bufs=1, name="fc")
                    ig = workp.tile([128, BL], fp16, tag="ig", bufs=1, name="ig")
                    tt = workp.tile([128, BL], fp16, tag="tt", name="tt")
                    ht = hpool.tile([128, BL], fp16, tag=f"h{hid}", name=f"h{hid}_{t}")
                    kp, sl = divmod(hid, 2)
                    if h8_next[kp] is None:
                        h8_next[kp] = hpool.tile(
                            [128, 2, BL], fp8, tag=f"p{kp}", name=f"p{kp}_{t}"
                        )
                    for s in halves:
                        nc.vector.tensor_tensor(
                            fc[:, s], gact[1][:, s], c_sb[hid][:, s], OP.mult
                        )
                        nc.vector.tensor_tensor(
                            ig[:, s], gact[0][:, s], gact[2][:, s], OP.mult
                        )
                        nc.vector.tensor_tensor(
                            c_sb[hid][:, s], fc[:, s], ig[:, s], OP.add
                        )
                        nc.scalar.activation(tt[:, s], c_sb[hid][:, s], AF.Tanh)
                        nc.vector.scalar_tensor_tensor(
                            h8_next[kp][:, sl, s], gact[3][:, s], SCL, tt[:, s],
                            OP.mult, OP.mult,
                        )
                        nc.vector.scalar_tensor_tensor(
                            ht[:, s], gact[3][:, s], SCL, tt[:, s], OP.mult, OP.mult
                        )
                    h_next[hid] = ht
                # y_{t-1} from h_cur (the h this step's matmuls consumed)
                y_tile(t - 1, h_cur)
                h_cur = h_next
                h8_cur = h8_next

            # final output y_{n-1} from the last h
            y_tile(n_steps - 1, h_cur)

    nc.compile()
    return nc


def _prepare_inputs(x, W1, b1, Wih, bih, Whh, bhh, W3, b3):
    """Host-side exact weight folding (fp64) + per-core sharding."""
    wih_col = Wih[:, 511:512].astype(np.float64)  # [2048,1]
    Whh_eff = Whh.astype(np.float64) + wih_col @ W3.astype(np.float64)  # [2048,512]
    bias_steady = (
        bih.astype(np.float64) + bhh.astype(np.float64) + wih_col[:, 0] * float(b3[0])
    )
    # full step-0 bias (applied to the RAW A psum, pre-scale)
    bias_d0 = bih.astype(np.float64) + bhh.astype(np.float64) - 0.8 * wih_col[:, 0]

    WT = (SCL * Whh_eff).T  # [512, 2048] scaled lhsT layout

    # fp8 DoubleRow weights for i/f/o m-tiles: [kp][p][slot][idx*128 + c]
    w8 = np.zeros((2, 128, 2, 12 * 128), np.float32)
    for idx, m in enumerate(M8):
        rows = slice(m * 128, (m + 1) * 128)
        for kp in range(2):
            for sl in range(2):
                k0 = kp * 256 + sl * 128
                w8[kp, :, sl, idx * 128 : (idx + 1) * 128] = WT[k0 : k0 + 128, rows]
    w8 = np.ascontiguousarray(w8).astype(FP8)

    # fp8 DoubleRow g-gate weights, k-pair 0 only (k rows 0-255)
    w8g = np.zeros((128, 2, 512), np.float32)
    for sl in range(2):
        w8g[:, sl, :] = WT[sl * 128 : (sl + 1) * 128, 1024:1536]
    w8g = np.ascontiguousarray(w8g).astype(FP8)

    # fp16 g-gate weights + unscaled W3 col (y = psum/16)
    wg = np.zeros((HID, 640), np.float64)
    wg[:, :512] = WT[:, 1024:1536]
    wg[:, 512] = W3[0].astype(np.float64)
    wg = wg.astype(np.float32).astype(FP16)

    we = np.zeros((HID, NG), np.float32)
    we[:EMB, :] = Wih[:, :EMB].T  # row 511 zero (emb row 511 is zero)

    w1t = np.zeros((24, 512), np.float32)
    w1t[:23, :EMB] = W1.T
    w1t[23, :EMB] = b1

    ident = np.eye(128, dtype=np.float32)

    bias_st_2d = bias_steady.reshape(M_TILES, 128).T.astype(np.float32)
    bias_d0_2d = bias_d0.reshape(M_TILES, 128).T.astype(np.float32)

    common = {
        "w1t": w1t.astype(FP16),
        "w8": w8,
        "w8g": w8g,
        "wg": wg,
        "we": we.astype(FP16),
        "ident": ident.astype(FP16),
        "bias_st": np.ascontiguousarray(bias_st_2d),
        "bias_d0": np.ascontiguousarray(bias_d0_2d),
    }
    in_maps = []
    for c in range(N_CORES):
        xs = x[c * BL : (c + 1) * BL]  # [BL, 23]
        xT = np.ones((24, BL), np.float32)
        xT[:23, :] = xs.T
        m = dict(common)
        m["xT"] = np.ascontiguousarray(xT).astype(FP16)
        in_maps.append(m)
    return in_maps, float(b3[0])


def kernel(x, W1, b1, Wih, bih, Whh, bhh, W3, b3):
    from concourse.bass_utils import run_bass_kernel_spmd

    in_maps, b3_val = _prepare_inputs(
        np.asarray(x, np.float32),
        np.asarray(W1, np.float32),
        np.asarray(b1, np.float32),
        np.asarray(Wih, np.float32),
        np.asarray(bih, np.float32),
        np.asarray(Whh, np.float32),
        np.asarray(bhh, np.float32),
        np.asarray(W3, np.float32),
        np.asarray(b3, np.float32),
    )
    nc = _build_program(b3_val)
    res = run_bass_kernel_spmd(nc, in_maps, list(range(N_CORES)))
    outs = [np.asarray(res.results[c]["out"]) for c in range(N_CORES)]  # [36, BL] each
    full = np.concatenate(outs, axis=1)  # [36, B]
    return full[:, :, None].astype(np.float32)  # [36, B, 1]


if __name__ == "__main__":
    rng = np.random.default_rng(0)
    ins = {
        "x": rng.standard_normal((B, 23), dtype=np.float32),
        "W1": rng.standard_normal((EMB, 23), dtype=np.float32) / np.sqrt(23),
        "b1": np.zeros(EMB, np.float32),
        "Wih": rng.standard_normal((NG, HID), dtype=np.float32) / np.sqrt(HID),
        "bih": np.zeros(NG, np.float32),
        "Whh": rng.standard_normal((NG, HID), dtype=np.float32) / np.sqrt(HID),
        "bhh": np.zeros(NG, np.float32),
        "W3": rng.standard_normal((1, HID), dtype=np.float32) / np.sqrt(HID),
        "b3": np.zeros(1, np.float32),
    }
    out = kernel(**ins)
    print("kernel output", out.shape, out.dtype, np.abs(out).max())


# revision 23
# speedup vs baseline: 1.5149x; 1.1989x over previous
"""Trainium2 Bass kernel for nn_ModelLSTM (36-step scalar-feedback LSTM).

Model (per reference):
    emb = relu(x @ W1.T + b1)                       # [B, 511], constant across steps
    x0 = -0.8; h0 = c0 = 0
    step t: inp = [emb, xin]                        # [B, 512]
            gates = inp @ Wih.T + bih + h @ Whh.T + bhh
            i,f,g,o = split(gates); c' = sig(f)*c + sig(i)*tanh(g); h' = sig(o)*tanh(c')
            y = h' @ W3.T + b3 ; xin' = y
    output ys = [36, B, 1]

Restructuring (host-side, exact in fp64):
  * xin folds into the recurrent weights: Whh_eff = Whh + Wih[:,511:] @ W3.
  * emb contribution A = Wih[:,:511] @ emb.T is precomputed once on device,
    kept RESIDENT in SBUF (fp16, x256 scale) and replayed into PSUM per step
    via an identity matmul (DVE-add on a couple of low-urgency tiles to
    balance PE vs DVE). Step-0 is fused into the A-precompute pass.

Mixed precision (validated in sim; ~0.016 rel err vs 2e-2 budget):
  * i/f/o gate matmuls: fp8 e4m3 x16-scaled weights AND h, DoubleRow perf
    mode (2 k-tiles per pass -> half the PE time; measured 214ns/MM).
  * g (tanh) gate matmul: fp8 DoubleRow for k rows 0-255, fp16 for rows
    256-511 (tanh slope 1 makes g 4x more error-sensitive; half-K fp8 keeps
    the error at ~sqrt(1/2) of full fp8).
  * Everything else fp16 (gates, h, c, A, emb); PSUM carries a 256x scale
    which ACT's free activation scale undoes.

Sharding: pure data-parallel over batch (16384 -> 8 cores x 2048). Weights
replicated. No collectives.
"""

import sys

for _p in ("/opt/trn_rl_repo",):
    if _p not in sys.path:
        sys.path.insert(0, _p)

import numpy as np
import ml_dtypes

BF16 = ml_dtypes.bfloat16
FP8 = ml_dtypes.float8_e4m3  # TRN variant (inf at 256); values are tiny
FP16 = np.float16

N_CORES = 8
B = 16384
BL = B // N_CORES  # 2048 batch per core
HID = 512
EMB = 511
STEPS = 36
NG = 4 * HID  # 2048 gate rows
M_TILES = 16  # gate row tiles of 128; m = 0..15, gate type = m//4, hid = m%4
K_TILES = 4  # contraction tiles of 128 over HID
NC_CH = BL // 512  # 4 free-dim chunks of 512
M8 = [0, 1, 2, 3, 4, 5, 6, 7, 12, 13, 14, 15]  # i/f/o m-tiles (full fp8 DoubleRow)
SCL = 16.0  # per-operand fp8/fp16 scale; psum carries SCL^2 = 256


def _build_program(
    b3_val: float,
    n_steps: int = STEPS,
    dve_add_ms: tuple = (6, 7),
    g_halfk: bool = True,
    split_hids: tuple = (0, 1),
    y_on_dve: bool = True,
):
    """Build the Bass program.

    dve_add_ms: m-tiles whose A contribution is added on DVE (psum + A -> z)
      instead of a PE identity matmul; balances PE vs DVE. Pick f-tiles of
      hids 2/3 (m=6,7): most slack before their h is consumed.
    g_halfk: g-gate matmul k rows 0-255 in fp8 DoubleRow, 256-511 in fp16.
    split_hids: hids whose cell update runs in column halves, so their h8
      lands early for the next step's first (kp=0) matmuls.
    """
    import concourse.bass as bass
    import concourse.bacc as bacc
    import concourse.tile as tile
    from concourse import mybir

    fp32 = mybir.dt.float32
    fp16 = mybir.dt.float16
    fp8 = mybir.dt.float8e4
    AF = mybir.ActivationFunctionType
    OP = mybir.AluOpType
    DR = mybir.MatmulPerfMode.DoubleRow

    nc = bacc.Bacc(
        "TRN2",
        target_bir_lowering=False,
        debug=False,
        num_devices=N_CORES,
    )

    # ---- DRAM I/O (per-core shapes) ----
    xT_d = nc.dram_tensor("xT", [24, BL], fp16, kind="ExternalInput")
    w1t_d = nc.dram_tensor("w1t", [24, 512], fp16, kind="ExternalInput")
    w8_d = nc.dram_tensor("w8", [2, 128, 2, 12 * 128], fp8, kind="ExternalInput")
    w8g_d = nc.dram_tensor("w8g", [128, 2, 512], fp8, kind="ExternalInput")
    wg_d = nc.dram_tensor("wg", [HID, 640], fp16, kind="ExternalInput")
    we_d = nc.dram_tensor("we", [HID, NG], fp16, kind="ExternalInput")
    ident_d = nc.dram_tensor("ident", [128, 128], fp16, kind="ExternalInput")
    bias_st_d = nc.dram_tensor("bias_st", [128, M_TILES], fp32, kind="ExternalInput")
    bias_d0_d = nc.dram_tensor("bias_d0", [128, M_TILES], fp32, kind="ExternalInput")
    out_d = nc.dram_tensor("out", [STEPS, BL], fp16, kind="ExternalOutput")

    with tile.TileContext(nc) as tc:
        with (
            tc.tile_pool(name="const", bufs=1) as constp,
            tc.tile_pool(name="apool", bufs=1) as apool,
            tc.tile_pool(name="state", bufs=1) as statep,
            tc.tile_pool(name="hpool", bufs=2) as hpool,
            tc.tile_pool(name="work", bufs=2) as workp,
            tc.tile_pool(name="psum", bufs=2, space=bass.MemorySpace.PSUM) as psump,
        ):
            # ---- load constants ----
            xT_sb = workp.tile([24, BL], fp16, tag="xT", bufs=1, name="xT_sb")
            nc.sync.dma_start(xT_sb[:], xT_d[:])
            w1t_sb = workp.tile([24, 512], fp16, tag="tt", name="w1t_sb")
            nc.sync.dma_start(w1t_sb[:], w1t_d[:])
            w8_sb = []
            for kp in range(2):
                wt = constp.tile([128, 2, 12 * 128], fp8, name=f"w8_{kp}")
                nc.sync.dma_start(wt[:], w8_d[kp])
                w8_sb.append(wt)
            w8g_sb = constp.tile([128, 2, 512], fp8, name="w8g")
            nc.sync.dma_start(w8g_sb[:], w8g_d[:])
            wg_sb = []
            for k in range(K_TILES):
                wt = constp.tile([128, 640], fp16, name=f"wg{k}")
                nc.sync.dma_start(wt[:], wg_d[k * 128 : (k + 1) * 128, :])
                wg_sb.append(wt)
            ident_sb = constp.tile([128, 128], fp16, name="ident")
            nc.sync.dma_start(ident_sb[:], ident_d[:])
            bias_st = constp.tile([128, M_TILES], fp32, name="bias_st")
            nc.sync.dma_start(bias_st[:], bias_st_d[:])
            bias_d0 = constp.tile([128, M_TILES], fp32, name="bias_d0")
            nc.sync.dma_start(bias_d0[:], bias_d0_d[:])

            # we k-tiles parked in the f-gate A slots (a4..a7) - those A tiles
            # are produced LAST in A_ORDER, and the a_stage hop decouples the
            # writeback so the slots recycle without deadlock.
            a_sb = [None] * M_TILES
            we_sb = []
            for k in range(K_TILES):
                wt = apool.tile([128, BL], fp16, tag=f"a{4 + k}", name=f"we{k}")
                nc.sync.dma_start(wt[:], we_d[k * 128 : (k + 1) * 128, :])
                we_sb.append(wt)

            # ---- setup: emb = relu(x @ W1.T + b1) (transposed, fp16) ----
            # emb k-tiles borrow the h16 tags (free during setup; step-0 h16
            # writes land in buf 1 of those tags).
            embT = []
            for mj in range(4):
                eps = psump.tile([128, BL], fp32, tag="gps", name="eps")
                for ncn in range(NC_CH):
                    s = slice(ncn * 512, (ncn + 1) * 512)
                    nc.tensor.matmul(
                        eps[:, s],
                        w1t_sb[:, mj * 128 : (mj + 1) * 128],
                        xT_sb[:, s],
                        start=True,
                        stop=True,
                    )
                et = hpool.tile([128, BL], fp16, tag=f"h{mj}", name=f"embT{mj}")
                nc.scalar.activation(et[:], eps[:], AF.Relu)
                embT.append(et)

            # ---- setup: A256[m] = 256 * We_m @ embT resident in SBUF (fp16),
            #      with step 0 fused in (gates0 = f(A_raw + bias_d0) from psum,
            #      c1 = i*g, h1 = o*tanh(c1)). ----
            h16 = [None] * K_TILES
            c_sb = [None] * K_TILES
            h8p = [None] * 2
            gact0 = {}
            STEP0_FUNC = {0: AF.Sigmoid, 2: AF.Tanh, 3: AF.Sigmoid}
            # hid-major i,g,o so step-0 cells finish early; f tiles last (f is
            # not needed in step 0, and their A slots hold `we` until the end).
            A_ORDER = [g * 4 + h for h in range(4) for g in (0, 2, 3)] + [4, 5, 6, 7]
            for m in A_ORDER:
                aps = psump.tile([128, BL], fp32, tag="gps", name="aps")
                for k in range(K_TILES):
                    for ncn in range(NC_CH):
                        s = slice(ncn * 512, (ncn + 1) * 512)
                        nc.tensor.matmul(
                            aps[:, s],
                            we_sb[k][:, m * 128 : (m + 1) * 128],
                            embT[k][:, s],
                            start=(k == 0),
                            stop=(k == K_TILES - 1),
                        )
                at = apool.tile([128, BL], fp16, tag=f"a{m}", name=f"a{m}")
                if 4 <= m <= 7:
                    # f-tile slots hold `we` until the end; stage + DMA hop
                    ast = workp.tile([128, BL], fp16, tag="z", name="astage")
                    nc.scalar.activation(ast[:], aps[:], AF.Copy, scale=SCL * SCL)
                    nc.sync.dma_start(at[:], ast[:])
                else:
                    nc.scalar.activation(at[:], aps[:], AF.Copy, scale=SCL * SCL)
                a_sb[m] = at
                gt, hid = divmod(m, 4)
                if gt in STEP0_FUNC:
                    g = workp.tile([128, BL], fp16, tag=f"g{gt}", name=f"g{gt}_0")
                    nc.scalar.activation(
                        g[:], aps[:], STEP0_FUNC[gt], bias=bias_d0[:, m : m + 1]
                    )
                    gact0[(gt, hid)] = g
                if gt == 3:
                    # o-gate staged: finish this hid's step-0 cell update
                    ct = statep.tile([128, BL], fp16, name=f"c{hid}")
                    nc.vector.tensor_tensor(
                        ct[:], gact0[(0, hid)][:], gact0[(2, hid)][:], OP.mult
                    )
                    tt = workp.tile([128, BL], fp16, tag="tt", name="t0")
                    nc.scalar.activation(tt[:], ct[:], AF.Tanh)
                    ht = hpool.tile([128, BL], fp16, tag=f"h{hid}", name=f"h{hid}_0")
                    nc.vector.scalar_tensor_tensor(
                        ht[:], gact0[(3, hid)][:], SCL, tt[:], OP.mult, OP.mult
                    )
                    h16[hid] = ht
                    c_sb[hid] = ct
                    kp, sl = divmod(hid, 2)
                    if h8p[kp] is None:
                        h8p[kp] = hpool.tile(
                            [128, 2, BL], fp8, tag=f"p{kp}", name=f"p{kp}_0"
                        )
                    nc.vector.tensor_copy(h8p[kp][:, sl, :], ht[:])
            h_cur = h16
            h8_cur = h8p

            # ---- steps 1..35 ----
            GATE_FUNC = {0: AF.Sigmoid, 1: AF.Sigmoid, 2: AF.Tanh, 3: AF.Sigmoid}
            inv = 1.0 / (SCL * SCL)

            def y_tile(t_out, h_in):
                """y = W3 @ (16h) / 16 (+ b3) -> out[t_out]."""
                gps = psump.tile([128, BL], fp32, tag="gps", name="yps")
                for k in range(K_TILES):
                    for ncn in range(NC_CH):
                        s = slice(ncn * 512, (ncn + 1) * 512)
                        nc.tensor.matmul(
                            gps[:, s],
                            wg_sb[k][:, 512:640],
                            h_in[k][:, s],
                            start=(k == 0),
                            stop=(k == K_TILES - 1),
                        )
                yr = workp.tile([1, BL], fp16, tag="yrow", bufs=1, name="yrow")
                if y_on_dve:
                    nc.vector.tensor_scalar(
                        yr[:], gps[0:1, :], 1.0 / SCL, float(b3_val), OP.mult, OP.add
                    )
                else:
                    nc.scalar.activation(
                        yr[:], gps[0:1, :], AF.Copy, bias=float(b3_val), scale=1.0 / SCL
                    )
                nc.sync.dma_start(out_d[t_out : t_out + 1, :], yr[:])

            for t in range(1, n_steps):
                h_next = [None] * K_TILES
                h8_next = [None] * 2
                for hid in range(K_TILES):
                    gact = {}
                    for gt in range(4):
                        m = gt * 4 + hid
                        use_dve = m in dve_add_ms
                        gps = psump.tile([128, BL], fp32, tag="gps", name="gps")
                        if not use_dve:
                            for ncn in range(NC_CH):
                                s = slice(ncn * 512, (ncn + 1) * 512)
                                nc.tensor.matmul(
                                    gps[:, s], ident_sb[:], a_sb[m][:, s],
                                    start=True, stop=False,
                                )
                        if gt == 2 and g_halfk:
                            # g: fp8 DoubleRow for k 0-255, fp16 for k 256-511
                            gc = slice(hid * 128, (hid + 1) * 128)
                            for ncn in range(NC_CH):
                                s = slice(ncn * 512, (ncn + 1) * 512)
                                nc.tensor.matmul(
                                    gps[:, s],
                                    w8g_sb[:, :, gc],
                                    h8_cur[0][:, :, s],
                                    start=use_dve,
                                    stop=False,
                                    perf_mode=DR,
                                )
                            for k in (2, 3):
                                for ncn in range(NC_CH):
                                    s = slice(ncn * 512, (ncn + 1) * 512)
                                    nc.tensor.matmul(
                                        gps[:, s],
                                        wg_sb[k][:, hid * 128 : (hid + 1) * 128],
                                        h_cur[k][:, s],
                                        start=False,
                                        stop=(k == K_TILES - 1),
                                    )
                        elif gt == 2:
                            for k in range(K_TILES):
                                for ncn in range(NC_CH):
                                    s = slice(ncn * 512, (ncn + 1) * 512)
                                    nc.tensor.matmul(
                                        gps[:, s],
                                        wg_sb[k][:, hid * 128 : (hid + 1) * 128],
                                        h_cur[k][:, s],
                                        start=(k == 0 and use_dve),
                                        stop=(k == K_TILES - 1),
                                    )
                        else:
                            # i/f/o: fp8 DoubleRow over 2 k-pairs
                            idx = M8.index(m)
                            mc = slice(idx * 128, (idx + 1) * 128)
                            for kp in range(2):
                                for ncn in range(NC_CH):
                                    s = slice(ncn * 512, (ncn + 1) * 512)
                                    nc.tensor.matmul(
                                        gps[:, s],
                                        w8_sb[kp][:, :, mc],
                                        h8_cur[kp][:, :, s],
                                        start=(kp == 0 and use_dve),
                                        stop=(kp == 1),
                                        perf_mode=DR,
                                    )
                        g = workp.tile([128, BL], fp16, tag=f"g{gt}", name=f"g{gt}_{t}")
                        if use_dve:
                            z = workp.tile([128, BL], fp16, tag="z", name="z")
                            nc.vector.tensor_tensor(z[:], gps[:], a_sb[m][:], OP.add)
                            nc.scalar.activation(
                                g[:], z[:], GATE_FUNC[gt],
                                bias=bias_st[:, m : m + 1], scale=inv,
                            )
                        else:
                            nc.scalar.activation(
                                g[:], gps[:], GATE_FUNC[gt],
                                bias=bias_st[:, m : m + 1], scale=inv,
                            )
                        gact[gt] = g
                    # cell update for this hid tile (all fp16 on DVE); h8 is
                    # written FIRST (it feeds the next step's matmuls), h16
                    # second (g-gate fp16 k-tiles + y only).
                    halves = (
                        [slice(0, BL // 2), slice(BL // 2, BL)]
                        if hid in split_hids
                        else [slice(0, BL)]
                    )
                    fc = workp.tile([128, BL], fp16, tag="fcig", bufs=1, name="fc")
                    ig = workp.tile([128, BL], fp16, tag="ig", bufs=1, name="ig")
                    tt = workp.tile([128, BL], fp16, tag="tt", name="tt")
                    ht = hpool.tile([128, BL], fp16, tag=f"h{hid}", name=f"h{hid}_{t}")
                    kp, sl = divmod(hid, 2)
                    if h8_next[kp] is None:
                        h8_next[kp] = hpool.tile(
                            [128, 2, BL], fp8, tag=f"p{kp}", name=f"p{kp}_{t}"
                        )
                    for s in halves:
                        nc.vector.tensor_tensor(
                            fc[:, s], gact[1][:, s], c_sb[hid][:, s], OP.mult
                        )
                        nc.vector.tensor_tensor(
                            ig[:, s], gact[0][:, s], gact[2][:, s], OP.mult
                        )
                        nc.vector.tensor_tensor(
                            c_sb[hid][:, s], fc[:, s], ig[:, s], OP.add
                        )
                        nc.scalar.activation(tt[:, s], c_sb[hid][:, s], AF.Tanh)
                        nc.vector.scalar_tensor_tensor(
                            h8_next[kp][:, sl, s], gact[3][:, s], SCL, tt[:, s],
                            OP.mult, OP.mult,
                        )
                        nc.vector.scalar_tensor_tensor(
                            ht[:, s], gact[3][:, s], SCL, tt[:, s], OP.mult, OP.mult
                        )
                    h_next[hid] = ht
                # y_{t-1} from h_cur (the h this step's matmuls consumed)
                y_tile(t - 1, h_cur)
                h_cur = h_next
                h8_cur = h8_next

            # final output y_{n-1} from the last h
            y_tile(n_steps - 1, h_cur)

    nc.compile()
    return nc


def _prepare_inputs(x, W1, b1, Wih, bih, Whh, bhh, W3, b3):
    """Host-side exact weight folding (fp64) + per-core sharding."""
    wih_col = Wih[:, 511:512].astype(np.float64)  # [2048,1]
    Whh_eff = Whh.astype(np.float64) + wih_col @ W3.astype(np.float64)  # [2048,512]
    bias_steady = (
        bih.astype(np.float64) + bhh.astype(np.float64) + wih_col[:, 0] * float(b3[0])
    )
    # full step-0 bias (applied to the RAW A psum, pre-scale)
    bias_d0 = bih.astype(np.float64) + bhh.astype(np.float64) - 0.8 * wih_col[:, 0]

    WT = (SCL * Whh_eff).T  # [512, 2048] scaled lhsT layout

    # fp8 DoubleRow weights for i/f/o m-tiles: [kp][p][slot][idx*128 + c]
    w8 = np.zeros((2, 128, 2, 12 * 128), np.float32)
    for idx, m in enumerate(M8):
        rows = slice(m * 128, (m + 1) * 128)
        for kp in range(2):
            for sl in range(2):
                k0 = kp * 256 + sl * 128
                w8[kp, :, sl, idx * 128 : (idx + 1) * 128] = WT[k0 : k0 + 128, rows]
    w8 = np.ascontiguousarray(w8).astype(FP8)

    # fp8 DoubleRow g-gate weights, k-pair 0 only (k rows 0-255)
    w8g = np.zeros((128, 2, 512), np.float32)
    for sl in range(2):
        w8g[:, sl, :] = WT[sl * 128 : (sl + 1) * 128, 1024:1536]
    w8g = np.ascontiguousarray(w8g).astype(FP8)

    # fp16 g-gate weights + unscaled W3 col (y = psum/16)
    wg = np.zeros((HID, 640), np.float64)
    wg[:, :512] = WT[:, 1024:1536]
    wg[:, 512] = W3[0].astype(np.float64)
    wg = wg.astype(np.float32).astype(FP16)

    we = np.zeros((HID, NG), np.float32)
    we[:EMB, :] = Wih[:, :EMB].T  # row 511 zero (emb row 511 is zero)

    w1t = np.zeros((24, 512), np.float32)
    w1t[:23, :EMB] = W1.T
    w1t[23, :EMB] = b1

    ident = np.eye(128, dtype=np.float32)

    bias_st_2d = bias_steady.reshape(M_TILES, 128).T.astype(np.float32)
    bias_d0_2d = bias_d0.reshape(M_TILES, 128).T.astype(np.float32)

    common = {
        "w1t": w1t.astype(FP16),
        "w8": w8,
        "w8g": w8g,
        "wg": wg,
        "we": we.astype(FP16),
        "ident": ident.astype(FP16),
        "bias_st": np.ascontiguousarray(bias_st_2d),
        "bias_d0": np.ascontiguousarray(bias_d0_2d),
    }
    in_maps = []
    for c in range(N_CORES):
        xs = x[c * BL : (c + 1) * BL]  # [BL, 23]
        xT = np.ones((24, BL), np.float32)
        xT[:23, :] = xs.T
        m = dict(common)
        m["xT"] = np.ascontiguousarray(xT).astype(FP16)
        in_maps.append(m)
    return in_maps, float(b3[0])


def kernel(x, W1, b1, Wih, bih, Whh, bhh, W3, b3):
    from concourse.bass_utils import run_bass_kernel_spmd

    in_maps, b3_val = _prepare_inputs(
        np.asarray(x, np.float32),
        np.asarray(W1, np.float32),
        np.asarray(b1, np.float32),
        np.asarray(Wih, np.float32),
        np.asarray(bih, np.float32),
        np.asarray(Whh, np.float32),
        np.asarray(bhh, np.float32),
        np.asarray(W3, np.float32),
        np.asarray(b3, np.float32),
    )
    nc = _build_program(b3_val)
    res = run_bass_kernel_spmd(nc, in_maps, list(range(N_CORES)))
    outs = [np.asarray(res.results[c]["out"]) for c in range(N_CORES)]  # [36, BL] each
    full = np.concatenate(outs, axis=1)  # [36, B]
    return full[:, :, None].astype(np.float32)  # [36, B, 1]


if __name__ == "__main__":
    rng = np.random.default_rng(0)
    ins = {
        "x": rng.standard_normal((B, 23), dtype=np.float32),
        "W1": rng.standard_normal((EMB, 23), dtype=np.float32) / np.sqrt(23),
        "b1": np.zeros(EMB, np.float32),
        "Wih": rng.standard_normal((NG, HID), dtype=np.float32) / np.sqrt(HID),
        "bih": np.zeros(NG, np.float32),
        "Whh": rng.standard_normal((NG, HID), dtype=np.float32) / np.sqrt(HID),
        "bhh": np.zeros(NG, np.float32),
        "W3": rng.standard_normal((1, HID), dtype=np.float32) / np.sqrt(HID),
        "b3": np.zeros(1, np.float32),
    }
    out = kernel(**ins)
    print("kernel output", out.shape, out.dtype, np.abs(out).max())
